# revision 1
# baseline (speedup 1.0000x reference)
"""Trainium2 Bass kernel for nn_Encoder (VGAE-style GNN encoder).

Computation (see reference):
  deg/norms from src/dst; h = relu(norm_dst * segsum_dst((feat*norm_src @ W1)[src]) + b1)
  agg2 = segsum_dst(h[src]);  mu = (agg2*norm_dst) @ W_mu + b_mu ; log_sigma likewise
  z = mu + noise * exp(log_sigma)

Strategy (graph/data parallel, dst-sharded, per the sharding hint):
  - nodes padded to NPAD, sharded SHARD per core; edges assigned to the core
    owning their dst node. Host does index preprocessing only: degree bincount
    -> norms, edge sort by (dst-supertile, src-window, dst) with 128-padded
    (supertile, window) groups, int16 gather index lists, per-block one-hot
    dst positions. The x1/h gather tables use a p-major row permutation so
    phase-1/epilogue stores are large-run group-batched DMAs (eidx absorbs
    the permutation).
  - phase 1: x1 = (featT stationary) @ W1 per 128-node tile; nsrc scale folded
    into the PSUM->SBUF copy; batched group stores -> AllGather f16 table
    (rows are 256B stride; only cols 0:H are ever fetched).
  - rounds: per (group, window) one dma_gather of 128B rows (elem_size=64
    f16 against the 256B row stride, emitted via raw_gather to bypass the
    transpose-only 256B elem restriction; pad slots fetch row 0 so message
    tiles never hold uninitialized SBUF). One batched DVE is_equal per
    (group, window) builds all one-hot [edge, dst] blocks at the f16 2x rate
    against an iota-replica tile with a stride-0 broadcast of dst positions;
    matmuls consume strided stationary slices of it. PSUM accumulators for
    all SB supertiles are packed into 2KB banks, DVE-memset once per group,
    and accumulated with start=False matmuls in window-arrival order (no
    open-chain interleave hazard, PE never waits for the last window).
  - epilogues: relu/scale on ACT into group staging tiles (h), and the
    mu/sigma branch with PE transposes, a merged mu/sig PSUM bank, exp on
    ACT, z = mu + noise*exp(ls) -> f16 group stores (host upcasts and
    unpermutes).
"""

import sys
import os
import numpy as np
from contextlib import ExitStack

if "/opt/trn_rl_repo" not in sys.path:
    sys.path.insert(0, "/opt/trn_rl_repo")

import concourse.bass as bass
import concourse.mybir as mybir
import concourse.tile as tile
from concourse.bacc import Bacc
from concourse.bass_utils import run_bass_kernel_spmd

F16 = mybir.dt.float16
F32 = mybir.dt.float32
I16 = mybir.dt.int16
ALU = mybir.AluOpType
ACTF = mybir.ActivationFunctionType

ST = 128  # supertile = dst nodes per PSUM accumulation tile


def raw_gather(gp, out_ap, in_ap, idxs_ap, num_idxs, num_idxs_reg, elem_size,
               elem_step, single_packet=False, queue_num=0):
    """dma_gather without the elem_size_bytes%256 assert (non-transpose, DRAM
    source, 256B-aligned row stride). Lets us fetch 128B (64xf16) rows from a
    256B-stride table, halving the per-descriptor DMA cost vs 256B fetches."""
    from concourse.ap_utils import ap_is_contiguous
    assert idxs_ap.dtype == mybir.dt.int16
    assert in_ap.dtype == out_ap.dtype
    assert ap_is_contiguous(in_ap.ap[1:])
    assert ap_is_contiguous(out_ap.ap[1:])
    assert ap_is_contiguous(idxs_ap.ap[1:])
    assert in_ap.ap[-1][1] == out_ap.ap[-1][1] == elem_size
    assert out_ap.ap[0][1] * out_ap.ap[1][1] == -(-num_idxs // 128) * 128
    assert in_ap.ap[0][0] == elem_step
    stride_bytes = elem_step * mybir.dt.size(in_ap.dtype)
    stride_bytes_256 = stride_bytes // 256
    assert stride_bytes % 256 == 0 and stride_bytes_256 < 256
    _in_ap = gp.lower_ap_dma(in_ap, for_custom_bir_dma=True)
    _idxs_ap = gp.lower_ap(idxs_ap)
    _out_ap = gp.lower_ap(out_ap)
    return gp.add_instruction(
        mybir.InstDMAGatherAnt(
            name=gp.bass.get_next_instruction_name(),
            ins=[*_in_ap, _idxs_ap, gp.lower_val_access(gp.to_reg(num_idxs_reg))],
            outs=[_out_ap],
            transpose=False,
            num_idxs=num_idxs,
            elem_size=elem_size,
            stride_bytes_256=stride_bytes_256,
            gen_mode=0,
            single_packet=single_packet,
            queue_num=queue_num,
            sbuf_tokens_per_rank=0,
            sbuf_free_dim_per_rank=0,
            sbuf_free_dim_pad_per_rank=0,
            sbuf_byte_offset=0,
        )
    )


def default_cfg(n, e, f, h):
    ncore = 8
    shard = -(-n // (ncore * ST)) * ST  # ceil to multiple of 128
    npad = shard * ncore
    nwin = 4
    win = -(-npad // nwin)
    assert win <= 32768, "int16 gather index range"
    nst = shard // ST
    # supertiles per gather group: largest divisor of nst keeping gathers
    # comfortably under the ~12800-idx SWDGE ring ceiling
    sb = 1
    for cand in range(1, nst + 1):
        if nst % cand == 0 and cand * 8 * 128 <= int(os.environ.get("KSBCAP", "7168")):
            sb = cand
    return dict(N=n, E=e, F=f, H=h, NCORE=ncore, SHARD=shard, NPAD=npad,
                NWIN=nwin, WIN=win, NST=nst, SB=sb)


def build_plan(src, dst, cfg):
    """Host-side index preprocessing. Returns per-core gather/one-hot arrays."""
    N, NCORE = cfg["N"], cfg["NCORE"]
    SHARD, NWIN, WIN, NST, SB = (cfg[k] for k in ("SHARD", "NWIN", "WIN", "NST", "SB"))
    src = np.asarray(src).astype(np.int64)
    dst = np.asarray(dst).astype(np.int64)

    core_of = dst // SHARD
    per_core = []
    cblk_need = 1
    for c in range(NCORE):
        sel = core_of == c
        s_c, d_c = src[sel], dst[sel]
        s_local = (d_c - c * SHARD) // ST
        # table row of src node: p-major within its owner shard
        srow = ((s_c // SHARD) * SHARD + (s_c % SHARD) % 128 * NST
                + (s_c % SHARD) // 128)
        w = srow // WIN
        order = np.lexsort((d_c, w, s_local))
        s_c, d_c, s_local, w = s_c[order], d_c[order], s_local[order], w[order]
        srow = srow[order]
        gid = s_local * NWIN + w
        cnt = np.bincount(gid, minlength=NST * NWIN)
        cblk_need = max(cblk_need, int(-(-cnt.max() // ST)))
        per_core.append((srow, d_c, gid, cnt, c))
    CBLK = int(cblk_need)
    GLEN = CBLK * ST                      # padded edges per (supertile, window) group
    NIDX = SB * CBLK * ST                 # idxs per gather instruction
    NCOLS = NIDX // 16                    # int16 idx columns per gather
    NG = NST // SB                        # gather groups per round
    NBLK = NST * NWIN * CBLK              # one-hot blocks per round

    plans = []
    for (srow, d_c, gid, cnt, c) in per_core:
        idx_flat = np.zeros(NST * NWIN * GLEN, dtype=np.int16)
        dloc_flat = np.full(NST * NWIN * GLEN, 300.0, dtype=np.float32)
        starts = np.concatenate(([0], np.cumsum(cnt)))
        # position of each edge inside the padded group layout
        pos = np.arange(len(srow)) - starts[gid] + gid * GLEN
        idx_flat[pos] = (srow % WIN).astype(np.int16)
        dloc_flat[pos] = (d_c - (c * SHARD + (gid // NWIN) * ST)).astype(np.float32)
        # eidx: per gather (g, w): concat si groups; wrap 16. The final
        # group's trailing pad slots become -1 (ucode skips their
        # descriptors); gcnt carries the per-gather valid count.
        eidx = np.zeros((128, NG * NWIN * NCOLS), dtype=np.int16)
        by_group = idx_flat.reshape(NST, NWIN, GLEN)
        for g in range(NG):
            for w in range(NWIN):
                # pad slots keep idx 0 (real fetch, zero one-hot row) so the
                # message tile never holds uninitialized (possibly-NaN) SBUF
                lst = by_group[g * SB:(g + 1) * SB, w, :].reshape(-1)
                wrapped = lst.reshape(NCOLS, 16).T  # [16, NCOLS]
                col0 = (g * NWIN + w) * NCOLS
                eidx[:, col0:col0 + NCOLS] = np.tile(wrapped, (8, 1))
        # dstloc: w-major col ((w*NST+s)*CBLK+k); partition p = edge k*128+p
        dstloc = (dloc_flat.reshape(NST, NWIN, CBLK, 128)
                  .transpose(1, 0, 2, 3)
                  .reshape(NWIN * NST * CBLK, 128).T
                  .astype(np.float16).copy())  # [128, NBLK]
        plans.append(dict(eidx=eidx, dstloc=dstloc))
    meta = dict(CBLK=CBLK, NIDX=NIDX, NCOLS=NCOLS, NG=NG, NBLK=NBLK,
                UPW=SB * CBLK)
    return plans, meta


def build_program(cfg, meta, sim_mode=False):
    NCORE, SHARD, NPAD = cfg["NCORE"], cfg["SHARD"], cfg["NPAD"]
    NWIN, WIN, NST, SB, F, H = (cfg[k] for k in ("NWIN", "WIN", "NST", "SB", "F", "H"))
    CBLK, NIDX, NCOLS, NG, NBLK = (meta[k] for k in ("CBLK", "NIDX", "NCOLS", "NG", "NBLK"))
    UPW = meta["UPW"]

    nc = Bacc(trn_type="TRN2", num_devices=NCORE)

    feat_tt = nc.dram_tensor("feat_tt", [F, SHARD], F16, kind="ExternalInput")
    nsrc = nc.dram_tensor("nsrc", [128, NST], F32, kind="ExternalInput")
    nprod = nc.dram_tensor("nprod", [128, NST], F32, kind="ExternalInput")
    ndst = nc.dram_tensor("ndst", [128, NST], F32, kind="ExternalInput")
    w1_16 = nc.dram_tensor("w1_16", [F, H], F16, kind="ExternalInput")
    wmu_16 = nc.dram_tensor("wmu_16", [H, H], F16, kind="ExternalInput")
    wsig_16 = nc.dram_tensor("wsig_16", [H, H], F16, kind="ExternalInput")
    b1_rep = nc.dram_tensor("b1_rep", [128, H], F32, kind="ExternalInput")
    bmu_col = nc.dram_tensor("bmu_col", [H, 1], F32, kind="ExternalInput")
    bsig_col = nc.dram_tensor("bsig_col", [H, 1], F32, kind="ExternalInput")
    eye16_d = nc.dram_tensor("eye16", [128, 128], F16, kind="ExternalInput")
    eye32_d = nc.dram_tensor("eye32", [H, H], F32, kind="ExternalInput")
    iota_rep_d = nc.dram_tensor("iota_rep", [128, 128, UPW], F16,
                                kind="ExternalInput")
    eidx_d = nc.dram_tensor("eidx", [128, NG * NWIN * NCOLS], I16, kind="ExternalInput")
    dstloc_d = nc.dram_tensor("dstloc", [128, NBLK], F16, kind="ExternalInput")
    noise_t = nc.dram_tensor("noise_t", [H, SHARD], F16, kind="ExternalInput")
    z_out = nc.dram_tensor("z_out", [H, SHARD], F16, kind="ExternalOutput")
    dbg = bool(int(os.environ.get("KDBG", "0")))
    if dbg:
        x1_dbg = nc.dram_tensor("x1_dbg", [128, NST, 128], F16,
                                kind="ExternalOutput")
        h_dbg = nc.dram_tensor("h_dbg", [128, NST, 128], F16,
                               kind="ExternalOutput")

    x1_shard = nc.dram_tensor("x1_shard", [128, NST, 128], F16, kind="Internal")
    h_shard = nc.dram_tensor("h_shard", [128, NST, 128], F16, kind="Internal")
    x1_table = nc.dram_tensor("x1_table", [NPAD, 128], F16, kind="Internal",
                              addr_space="Shared")
    h_table = nc.dram_tensor("h_table", [NPAD, 128], F16, kind="Internal",
                             addr_space="Shared")
    groups = [list(range(NCORE))]

    with tile.TileContext(nc) as tc, ExitStack() as ctx:
        consts = ctx.enter_context(tc.tile_pool(name="consts", bufs=1))

        def cload(dram, shape, dtype, tag):
            t = consts.tile(shape, dtype, tag=tag)
            nc.sync.dma_start(t[:], dram[:])
            return t

        w1_sb = cload(w1_16, [F, H], F16, "w1")
        nsrc_sb = cload(nsrc, [128, NST], F32, "nsrc")

        # ---------------- phase 1: x1 = (feat*nsrc) @ W1 on own shard -------
        with tc.tile_pool(name="p1", bufs=4) as p1, \
             tc.tile_pool(name="p1ps", bufs=8, space="PSUM") as p1ps:
            for g in range(NG):
                ftg = p1.tile([F, SB * 128], F16, tag="ftg")
                nc.sync.dma_start(ftg[:],
                                  feat_tt[:, g * SB * 128:(g + 1) * SB * 128])
                xg = p1.tile([128, SB, 128], F16, tag="xg")
                for si in range(SB):
                    t = g * SB + si
                    x1p = p1ps.tile([128, H], F32, tag="x1p")
                    nc.tensor.matmul(x1p[:], ftg[:, si * 128:(si + 1) * 128],
                                     w1_sb[:], start=True, stop=True)
                    nc.vector.tensor_scalar(xg[:, si, 0:H], x1p[:],
                                            nsrc_sb[:, t:t + 1], None, ALU.mult)
                nc.sync.dma_start(x1_shard[:, g * SB:(g + 1) * SB, :], xg[:])

        # round-only consts: loaded after phase-1 emission so the feat /
        # x1 DMAs aren't queued behind them
        wmu_sb = cload(wmu_16, [H, H], F16, "wmu")
        wsig_sb = cload(wsig_16, [H, H], F16, "wsig")
        ndst_sb = cload(ndst, [128, NST], F32, "ndst")
        nprod_sb = cload(nprod, [128, NST], F32, "nprod")
        b1_sb = cload(b1_rep, [128, H], F32, "b1")
        bmu_sb = cload(bmu_col, [H, 1], F32, "bmu")
        bsig_sb = cload(bsig_col, [H, 1], F32, "bsig")
        eye16 = cload(eye16_d, [128, 128], F16, "eye16")
        iota_rep = cload(iota_rep_d, [128, 128, UPW], F16, "iota_rep")
        dstloc_sb = cload(dstloc_d, [128, NBLK], F16, "dstloc")
        eidx_sb = cload(eidx_d, [128, NG * NWIN * NCOLS], I16, "eidx")

        if dbg:
            nc.sync.dma_start(x1_dbg[:], x1_shard[:, :, :])
        if sim_mode:
            nc.sync.dma_start(x1_table[0:SHARD, :], x1_shard[:, :, :])
        else:
            nc.gpsimd.collective_compute("AllGather", ALU.bypass, groups,
                                         ins=[x1_shard[:]], outs=[x1_table[:]])

        # ---------------- message-passing round ----------------------------
        def mp_round(table, epilogue, pre_group, post_group):
            with tc.tile_pool(name="msgs", bufs=2) as msgs, \
                 tc.tile_pool(name="ohp", bufs=(7 if UPW <= 40 else 3)) as ohp, \
                 tc.tile_pool(name="aggps", bufs=3, space="PSUM") as aggps, \
                 tc.tile_pool(name="epi", bufs=3) as epi, \
                 tc.tile_pool(name="episb", bufs=3) as episb, \
                 tc.tile_pool(name="stg", bufs=6) as stg, \
                 tc.tile_pool(name="epips", bufs=2, space="PSUM") as epips, \
                 tc.tile_pool(name="epips2", bufs=2, space="PSUM") as epips2:
                for g in range(NG):
                    # emit window 0 last: it is the only window whose table
                    # rows overlap the own-shard copy, and Pool's in-order
                    # sequencer would head-of-line-block windows 1-3 behind
                    # that dependency at the round boundary
                    mt = {}
                    for w in (1, 2, 3, 0):
                        m = msgs.tile([128, SB * CBLK, H], F16, tag=f"m{w}")
                        col0 = (g * NWIN + w) * NCOLS
                        raw_gather(
                            nc.gpsimd, m[:], table[w * WIN:(w + 1) * WIN, 0:H],
                            eidx_sb[:, col0:col0 + NCOLS],
                            num_idxs=NIDX, num_idxs_reg=NIDX, elem_size=H,
                            elem_step=128, single_packet=False)
                        mt[w] = m
                    # group staging/loads emitted after the gathers so their
                    # DMAs queue behind them on the FIFO DMA device
                    ctxg = pre_group(g, stg, epi)
                    # batched one-hots per window (DVE; independent of gathers)
                    ohs = []
                    for w in range(NWIN):
                        col0 = (w * NST + g * SB) * CBLK
                        oh = ohp.tile([128, 128, UPW], F16, tag="oh")
                        nc.vector.tensor_tensor(
                            oh[:], iota_rep[:],
                            dstloc_sb[:, None, col0:col0 + UPW]
                            .broadcast_to([128, 128, UPW]),
                            ALU.is_equal)
                        ohs.append(oh)
                    # pack up to 7 supertile accumulators per 2KB PSUM bank;
                    # memset + start=False accumulate-mode matmuls let windows
                    # be consumed in arrival order (no open-chain interleave
                    # hazard, no wait-for-last-window before PE starts)
                    banks = []
                    for hb in range(-(-SB // 7)):
                        nsi = min(7, SB - hb * 7)
                        ab = aggps.tile([128, nsi * H], F32, tag=f"ab{hb}")
                        nc.vector.memset(ab[:], 0.0)
                        banks.append(ab)
                    for w in range(NWIN):
                        for si in range(SB):
                            agg = banks[si // 7][:, (si % 7) * H:(si % 7 + 1) * H]
                            for k in range(CBLK):
                                u = si * CBLK + k
                                nc.tensor.matmul(
                                    agg, ohs[w][:, :, u], mt[w][:, u, 0:H],
                                    start=False,
                                    stop=(w == NWIN - 1 and k == CBLK - 1))
                    for si in range(SB):
                        agg = banks[si // 7][:, (si % 7) * H:(si % 7 + 1) * H]
                        epilogue(g, si, agg, ctxg, epi, episb, epips, epips2)
                    post_group(g, ctxg)

        def pre_r1(g, stg, epi):
            hg = stg.tile([128, SB, 128], F16, tag="hg")
            return dict(hg=hg)

        def post_r1(g, ctxg):
            nc.sync.dma_start(h_shard[:, g * SB:(g + 1) * SB, :], ctxg["hg"][:])

        def epi_round1(g, si, agg, ctxg, epi, episb, epips, epips2):
            s = g * SB + si
            if cfg.get("B1Z", True):
                # b1 == 0 and nsrc > 0, so relu(ndst*agg)*nsrc ==
                # relu(ndst*nsrc*agg): one ACT op with the combined scale
                nc.scalar.activation(ctxg["hg"][:, si, 0:H], agg, ACTF.Relu,
                                     scale=nprod_sb[:, s:s + 1])
            else:
                hp = epi.tile([128, H], F32, tag="hp")
                nc.vector.scalar_tensor_tensor(hp[:], agg, ndst_sb[:, s:s + 1],
                                               b1_sb[:], ALU.mult, ALU.add)
                nc.scalar.activation(ctxg["hg"][:, si, 0:H], hp[:], ACTF.Relu,
                                     scale=nsrc_sb[:, s:s + 1])

        def pre_r2(g, stg, epi):
            zg = stg.tile([H, SB * 128], F16, tag="zg")
            ng = stg.tile([H, SB * 128], F16, tag="ng")
            nc.sync.dma_start(ng[:], noise_t[:, g * SB * 128:(g + 1) * SB * 128])
            return dict(zg=zg, ng=ng)

        def post_r2(g, ctxg):
            nc.sync.dma_start(z_out[:, g * SB * 128:(g + 1) * SB * 128],
                              ctxg["zg"][:])

        def epi_round2(g, si, agg, ctxg, epi, episb, epips, epips2):
            s = g * SB + si
            a2s = epi.tile([128, H], F16, tag="a2s")
            nc.scalar.activation(a2s[:], agg, ACTF.Identity,
                                 scale=ndst_sb[:, s:s + 1])
            a2tp = epips.tile([H, 128], F16, tag="a2tp")
            nc.tensor.matmul(a2tp[:], a2s[:], eye16[:], is_transpose=True)
            a2t = epi.tile([H, 128], F16, tag="a2t")
            nc.scalar.activation(a2t[:], a2tp[:], ACTF.Identity)
            musg = epips2.tile([H, 2, 128], F32, tag="musg")
            nc.tensor.matmul(musg[:, 0, :], wmu_sb[:], a2t[:], start=True,
                             stop=True)
            nc.tensor.matmul(musg[:, 1, :], wsig_sb[:], a2t[:], start=True,
                             stop=True)
            mub = episb.tile([H, 128], F32, tag="mub")
            nc.scalar.activation(mub[:], musg[:, 0, :], ACTF.Identity,
                                 bias=bmu_sb[:])
            es = episb.tile([H, 128], F32, tag="es")
            nc.scalar.activation(es[:], musg[:, 1, :], ACTF.Exp,
                                 bias=bsig_sb[:])
            nz = episb.tile([H, 128], F32, tag="nz")
            nc.vector.scalar_tensor_tensor(
                nz[:], ctxg["ng"][:, si * 128:(si + 1) * 128], 1.0, es[:],
                ALU.mult, ALU.mult)
            nc.vector.scalar_tensor_tensor(
                ctxg["zg"][:, si * 128:(si + 1) * 128], mub[:], 0.0, nz[:],
                ALU.add, ALU.add)

        kphase = int(os.environ.get("KPHASE", "4"))
        if kphase >= 2:
            mp_round(x1_table, epi_round1, pre_r1, post_r1)
        if kphase >= 3:
            if dbg:
                nc.sync.dma_start(h_dbg[:], h_shard[:, :, :])
            if sim_mode:
                nc.sync.dma_start(h_table[0:SHARD, :], h_shard[:, :, :])
            else:
                nc.gpsimd.collective_compute("AllGather", ALU.bypass, groups,
                                             ins=[h_shard[:, :, :]],
                                             outs=[h_table[:]])
        if kphase >= 4:
            mp_round(h_table, epi_round2, pre_r2, post_r2)

    nc.finalize()
    return nc


def host_inputs(feat, src, dst, noise, W1, b1, W_mu, b_mu, W_sig, b_sig,
                cfg, plans):
    N, NCORE, SHARD, NPAD = (cfg[k] for k in ("N", "NCORE", "SHARD", "NPAD"))
    NST, F, H = cfg["NST"], cfg["F"], cfg["H"]
    feat = np.asarray(feat, dtype=np.float32)
    noise = np.asarray(noise, dtype=np.float32)
    src = np.asarray(src); dst = np.asarray(dst)

    deg_out = np.bincount(src, minlength=NPAD).astype(np.float32)
    deg_in = np.bincount(dst, minlength=NPAD).astype(np.float32)
    norm_src = np.maximum(deg_out, 1.0) ** -0.5
    norm_dst = np.maximum(deg_in, 1.0) ** -0.5

    featp = np.zeros((NPAD, F), dtype=np.float32)
    featp[:N] = feat
    noisep = np.zeros((NPAD, H), dtype=np.float32)
    noisep[:N] = noise

    eye16 = np.eye(128, dtype=np.float16)
    eye32 = np.eye(H, dtype=np.float32)
    UPW = cfg["SB"] * plans[0]["dstloc"].shape[1] // (cfg["NST"] * cfg["NWIN"])
    iota_rep = np.tile(np.arange(128, dtype=np.float16)[None, :, None],
                       (128, 1, UPW))
    shared = dict(
        w1_16=np.asarray(W1, dtype=np.float16),
        wmu_16=np.asarray(W_mu, dtype=np.float16),
        wsig_16=np.asarray(W_sig, dtype=np.float16),
        b1_rep=np.tile(np.asarray(b1, dtype=np.float32)[None, :], (128, 1)),
        bmu_col=np.asarray(b_mu, dtype=np.float32).reshape(H, 1),
        bsig_col=np.asarray(b_sig, dtype=np.float32).reshape(H, 1),
        eye16=eye16, eye32=eye32, iota_rep=iota_rep,
    )
    in_maps = []
    for c in range(NCORE):
        lo, hi = c * SHARD, (c + 1) * SHARD
        m = dict(shared)
        m["feat_tt"] = featp[lo:hi].T.astype(np.float16).copy()
        m["nsrc"] = norm_src[lo:hi].reshape(NST, 128).T.copy()
        m["nprod"] = (norm_src * norm_dst)[lo:hi].reshape(NST, 128).T.copy()
        m["ndst"] = norm_dst[lo:hi].reshape(NST, 128).T.copy()
        m["noise_t"] = noisep[lo:hi].T.astype(np.float16).copy()
        m["eidx"] = plans[c]["eidx"]
        m["dstloc"] = plans[c]["dstloc"]
        in_maps.append(m)
    return in_maps


def run(feat, src, dst, noise, W1, b1, W_mu, b_mu, W_sig, b_sig,
        cfg=None, **spmd_kwargs):
    if cfg is None:
        cfg = default_cfg(feat.shape[0], src.shape[0], feat.shape[1],
                          W1.shape[1])
    cfg["B1Z"] = bool(np.all(np.asarray(b1) == 0.0))
    plans, meta = build_plan(src, dst, cfg)
    nc = build_program(cfg, meta)
    in_maps = host_inputs(feat, src, dst, noise, W1, b1, W_mu, b_mu,
                          W_sig, b_sig, cfg, plans)
    import time as _time
    last_exc = None
    for attempt in range(3):
        try:
            res = run_bass_kernel_spmd(nc, in_maps,
                                       core_ids=list(range(cfg["NCORE"])),
                                       **spmd_kwargs)
            break
        except Exception as e:  # transient NRT device errors: retry
            last_exc = e
            _time.sleep(10.0)
    else:
        raise last_exc
    zs = [r["z_out"].T for r in res.results]  # [H, SHARD] feature-major
    z = np.concatenate(zs, axis=0)[:cfg["N"]]
    return z.astype(np.float32), res


def kernel(feat, src, dst, noise, W1, b1, W_mu, b_mu, W_sig, b_sig):
    z, _ = run(feat, src, dst, noise, W1, b1, W_mu, b_mu, W_sig, b_sig)
    return z



# revision 33
# speedup vs baseline: 1.3146x; 1.3146x over previous
"""Trainium2 Bass kernel for nn_Encoder (VGAE-style GNN encoder).

Computation (see reference):
  deg/norms from src/dst; h = relu(ndst * segsum_dst((feat*nsrc @ W1)[src]))
  agg2 = segsum_dst((h*nsrc)[src]); mu = (agg2*ndst) @ W_mu + b_mu ; ls likewise
  z = mu + noise * exp(log_sigma)

Strategy (graph/data parallel, dst-sharded, K-grid + one-hot-spill reduce):
  - Nodes globally sorted by in-degree and striped across (core, supertile)
    so every core sees the same degree profile (the SPMD program is shared).
    Host pre-permutes feat/noise/norms into table order, un-permutes z.
  - Message tables (x1, h) stored fp8e4 in 256B-stride rows (64B payload):
    dma_gather descriptors hit the 7ns cost floor. 4 src windows (int16 idx).
  - Per (supertile, window): a K-deep grid where slot (k, p) holds the k-th
    window-w in-edge of dst slot p (pads fetch a zero/fake row). The reduce is
    a PSUM-accumulated matmul with a single shared fp8 identity stationary --
    one matmul per (window, k, 7-supertile bank half), no per-block Ldweights
    and no per-edge DVE one-hot work. Edges beyond the grid depth go through
    a small spill path using classic one-hot (f16 stationary x fp8 moving)
    blocks. K is host-optimized per (supertile, window) to minimize total
    gather slots; serpentine supertile->group dealing equalizes SBUF tiles.
  - h staged fully in SBUF, one strided store per round; z/noise staged whole.
  - Epilogues as before: relu/scale on ACT (nprod = nsrc*ndst folds both the
    round-1 post-scale and round-2 pre-scale); mu/sig branch with PE
    transposes, merged mu/sig PSUM, exp on ACT, z = mu + noise*exp(ls).
"""

import os
import sys
import numpy as np
from contextlib import ExitStack

if "/opt/trn_rl_repo" not in sys.path:
    sys.path.insert(0, "/opt/trn_rl_repo")

import concourse.bass as bass
import concourse.mybir as mybir
import concourse.tile as tile
from concourse.bacc import Bacc
from concourse.bass_utils import run_bass_kernel_spmd

F16 = mybir.dt.float16
F32 = mybir.dt.float32
F8 = mybir.dt.float8e4
I16 = mybir.dt.int16
ALU = mybir.AluOpType
ACTF = mybir.ActivationFunctionType

ST = 128


def raw_gather(gp, out_ap, in_ap, idxs_ap, num_idxs, num_idxs_reg, elem_size,
               elem_step, single_packet=False, queue_num=0):
    """dma_gather without the elem_size_bytes%256 assert (non-transpose, DRAM
    source, 256B-aligned row stride): fetches sub-row payloads (64B fp8 /
    128B f16) from a 256B-stride table."""
    from concourse.ap_utils import ap_is_contiguous
    assert idxs_ap.dtype == mybir.dt.int16
    assert in_ap.dtype == out_ap.dtype
    assert ap_is_contiguous(in_ap.ap[1:])
    assert ap_is_contiguous(out_ap.ap[1:])
    assert ap_is_contiguous(idxs_ap.ap[1:])
    assert in_ap.ap[-1][1] == out_ap.ap[-1][1] == elem_size
    assert out_ap.ap[0][1] * out_ap.ap[1][1] == -(-num_idxs // 128) * 128
    assert in_ap.ap[0][0] == elem_step
    stride_bytes = elem_step * mybir.dt.size(in_ap.dtype)
    stride_bytes_256 = stride_bytes // 256
    assert stride_bytes % 256 == 0 and stride_bytes_256 < 256
    _in_ap = gp.lower_ap_dma(in_ap, for_custom_bir_dma=True)
    _idxs_ap = gp.lower_ap(idxs_ap)
    _out_ap = gp.lower_ap(out_ap)
    return gp.add_instruction(
        mybir.InstDMAGatherAnt(
            name=gp.bass.get_next_instruction_name(),
            ins=[*_in_ap, _idxs_ap, gp.lower_val_access(gp.to_reg(num_idxs_reg))],
            outs=[_out_ap],
            transpose=False,
            num_idxs=num_idxs,
            elem_size=elem_size,
            stride_bytes_256=stride_bytes_256,
            gen_mode=0,
            single_packet=single_packet,
            queue_num=queue_num,
            sbuf_tokens_per_rank=0,
            sbuf_free_dim_per_rank=0,
            sbuf_free_dim_pad_per_rank=0,
            sbuf_byte_offset=0,
        )
    )


def default_cfg(n, e, f, h):
    ncore = 8
    shard = -(-n // (ncore * ST)) * ST
    npad = shard * ncore
    nst = shard // ST
    nwin = 4
    win = -(-npad // nwin)  # p-major table rows per window
    assert win <= 32768, "int16 gather index range"
    sb = int(os.environ.get("KSB", "14"))
    while nst % sb:
        sb -= 1
    return dict(N=n, E=e, F=f, H=h, NCORE=ncore, SHARD=shard, NPAD=npad,
                NWIN=nwin, WIN=win, NST=nst, SB=sb, NG=nst // sb)


def _serpentine_deal(nst, ng, sb):
    """deal_to_st[d] = table supertile index (g*sb + j) of the d-th
    degree-ranked supertile: snake-deal ranks across groups so per-group
    row totals stay balanced while j stays degree-descending in each group."""
    deal_to_st = np.empty(nst, dtype=np.int64)
    d = 0
    for p in range(sb):
        order = range(ng) if p % 2 == 0 else range(ng - 1, -1, -1)
        for gi in order:
            deal_to_st[d] = gi * sb + p
            d += 1
    return deal_to_st


def build_plan(src, dst, cfg):
    """Host-side index preprocessing. Returns per-core arrays + uniform meta."""
    N, NCORE = cfg["N"], cfg["NCORE"]
    SHARD, NPAD, NWIN, WIN, NST, SB, NG = (
        cfg[k] for k in ("SHARD", "NPAD", "NWIN", "WIN", "NST", "SB", "NG"))
    src = np.asarray(src).astype(np.int64)
    dst = np.asarray(dst).astype(np.int64)

    deg_in = np.bincount(dst, minlength=NPAD)
    # global in-degree sort; stripe supertiles across cores so all cores share
    # one degree profile (SPMD program uniformity)
    order = np.argsort(-deg_in, kind="stable")
    # spread the fake (zero) nodes across the last 8 global supertiles (one
    # per core) so every gather window contains a zero row for pad slots
    nfake = NPAD - N
    if nfake:
        tailn = max(1024, -(-nfake // 8) * 8 * 2)
        tailn = min(tailn, NPAD)
        tail = order[-tailn:].copy()
        fk = tail[tail >= N]
        rl = tail[tail < N]
        newtail = np.empty_like(tail)
        taken = np.zeros(tailn, dtype=bool)
        nst_tail = min(8, tailn // ST)
        for j, f in enumerate(fk):
            o2 = (j % nst_tail) * ST + j // nst_tail
            newtail[o2] = f
            taken[o2] = True
        newtail[~taken] = rl
        order[-tailn:] = newtail
    rank = np.arange(NPAD)
    sti = rank // ST                      # global sorted supertile 0..NST*8-1
    st_core = sti % NCORE
    deal_to_st = _serpentine_deal(NST, NG, SB)
    st_in_core = deal_to_st[sti // NCORE]
    posn = st_core * SHARD + st_in_core * ST + (rank % ST)
    pos_of = np.empty(NPAD, dtype=np.int64)
    pos_of[order] = posn

    # p-major table row of a position (matches phase-1 store layout)
    def srow_of(pos):
        return (pos // SHARD) * SHARD + (pos % SHARD) % ST * NST + (pos % SHARD) // ST

    srow_all = srow_of(np.arange(NPAD))
    # zero-pad target row per window: a fake position (node id >= N) in range
    fake_pos = pos_of[N:] if NPAD > N else None
    zpad = np.zeros(NWIN, dtype=np.int64)
    if fake_pos is not None and len(fake_pos):
        fr = np.sort(srow_of(fake_pos))
        for w in range(NWIN):
            cand = fr[(fr >= w * WIN) & (fr < (w + 1) * WIN)]
            assert len(cand), f"no fake row in window {w}"
            zpad[w] = cand[0]
    groups = [[g * SB + j for j in range(SB)] for g in range(NG)]

    dstp = pos_of[dst]
    srcp = pos_of[src]
    srow = srow_of(srcp)
    wofe = srow // WIN
    core = dstp // SHARD
    stl = (dstp % SHARD) // ST
    slot = dstp % ST

    # per-core per-(st,w) degree per slot
    deg = np.zeros((NCORE, NST, NWIN, ST), dtype=np.int32)
    key = ((core * NST + stl) * NWIN + wofe) * ST + slot
    cnts = np.bincount(key, minlength=NCORE * NST * NWIN * ST)
    deg = cnts.reshape(NCORE, NST, NWIN, ST)

    # K optimization per (st, w), SPMD-uniform across cores:
    # cost(K) = 128*K + 128*max_c ceil(spill_c(K)/128)
    degM = deg  # [C, NST, NWIN, ST]
    maxdeg = int(degM.max())
    # a spill block costs more than a grid row (DVE one-hot build + per-block
    # Ldweights/matmul issue vs one batched identity matmul)
    spill_w = float(os.environ.get("KSPW", "1.7"))
    K = np.zeros((NST, NWIN), dtype=np.int32)
    SPB = np.zeros((NST, NWIN), dtype=np.int32)
    for s in range(NST):
        for w in range(NWIN):
            d = degM[:, s, w, :]  # [C, ST]
            best, bestk, bestspb = None, 0, 0
            for k in range(0, min(maxdeg, int(d.max())) + 1):
                spill = np.maximum(d - k, 0).sum(axis=1).max()
                spb = -(-int(spill) // ST)
                c = ST * k + spill_w * ST * spb
                if best is None or c < best:
                    best, bestk, bestspb = c, k, spb
            K[s, w], SPB[s, w] = bestk, bestspb
    # monotone non-increasing K along each group's st order (prefix matmuls)
    for g in range(NG):
        sts = groups[g]
        for w in range(NWIN):
            for j in range(SB - 2, -1, -1):
                K[sts[j], w] = max(K[sts[j], w], K[sts[j + 1], w])
    # recompute spill block counts at the final K
    for s in range(NST):
        for w in range(NWIN):
            spill = np.maximum(degM[:, s, w, :] - K[s, w], 0).sum(axis=1).max()
            SPB[s, w] = -(-int(spill) // ST)

    # uniform row layout per (g, w): grid rows k-major, then spill rows
    rows_gw = np.zeros((NG, NWIN), dtype=np.int64)
    grid_layout = {}   # (g,w) -> list of (k, n_k, row_start)
    spill_layout = {}  # (g,w) -> list of (j, st, row_start, nblocks)
    for g in range(NG):
        sts = groups[g]
        for w in range(NWIN):
            r = 0
            gl = []
            kmax = int(K[sts[0], w])
            for k in range(kmax):
                nk = int(sum(1 for s in sts if K[s, w] > k))
                gl.append((k, nk, r))
                r += nk
            sl = []
            bloc = 0
            for j, s in enumerate(sts):
                nb = int(SPB[s, w])
                if nb:
                    sl.append((j, s, r, nb, bloc))
                    r += nb
                    bloc += nb
            grid_layout[(g, w)] = gl
            spill_layout[(g, w)] = sl
            rows_gw[g, w] = r
    ROWS_CAP = int(rows_gw.max())
    # spill block base offsets in dstloc, ordered (g, w); NBMAX = per-(g,w) max
    spb_off = {}
    acc = 0
    NBMAX = 1
    for g in range(NG):
        for w in range(NWIN):
            spb_off[(g, w)] = acc
            nb_gw = sum(nb for (_, _, _, nb, _) in spill_layout[(g, w)])
            NBMAX = max(NBMAX, nb_gw)
            acc += nb_gw
    NSPB = acc
    # eidx col offsets per (g, w)
    col_off = {}
    acc = 0
    for g in range(NG):
        for w in range(NWIN):
            col_off[(g, w)] = acc
            acc += int(rows_gw[g, w]) * (ST // 16)
    TOTCOLS = acc

    # per-core edge placement
    plans = []
    for c in range(NCORE):
        sel = core == c
        e_st, e_w, e_slot = stl[sel], wofe[sel], slot[sel]
        e_srow = srow[sel]
        o = np.lexsort((e_srow, e_slot, e_w, e_st))
        e_st, e_w, e_slot, e_srow = e_st[o], e_w[o], e_slot[o], e_srow[o]
        # rank of edge within its (st, w, slot) list
        key2 = (e_st * NWIN + e_w) * ST + e_slot
        # edges sorted by key2 groups (lexsort above ensures grouping)
        o2 = np.argsort(key2, kind="stable")
        k2s = key2[o2]
        within = np.arange(len(k2s)) - np.searchsorted(k2s, k2s, side="left")
        e_k = np.empty(len(k2s), dtype=np.int64)
        e_k[o2] = within

        eidx = np.zeros((128, TOTCOLS), dtype=np.int16)
        dloc = np.full((128, max(NSPB, 1)), 300.0, dtype=np.float16)
        st_j = {}
        for g in range(NG):
            for j, s in enumerate(groups[g]):
                st_j[s] = (g, j)
        # grid placement
        for g in range(NG):
            sts = groups[g]
            jmap = -np.ones(NST, dtype=np.int64)
            for j, s in enumerate(sts):
                jmap[s] = j
            for w in range(NWIN):
                r_gw = int(rows_gw[g, w])
                if r_gw == 0:
                    continue
                idx_flat = np.full(r_gw * ST, zpad[w] - w * WIN, dtype=np.int64)
                # grid rows
                gsel = (np.isin(e_st, sts) & (e_w == w)
                        & (e_k < K[e_st, w]))
                gs = np.nonzero(gsel)[0]
                if len(gs):
                    kk = e_k[gs]
                    ss = e_st[gs]
                    jj = jmap[ss]
                    # row of (k, st j): row_start(k) + position of j among
                    # sts with K> k (prefix since K monotone in j)
                    gl = grid_layout[(g, w)]
                    rstart = np.zeros(int(K[sts[0], w]) + 1, dtype=np.int64)
                    for (k, nk, rs) in gl:
                        rstart[k] = rs
                    rows = rstart[kk] + jj
                    idx_flat[rows * ST + e_slot[gs]] = e_srow[gs] - w * WIN
                # spill rows
                for (j, s, rs, nb, bloc) in spill_layout[(g, w)]:
                    ssel = np.nonzero((e_st == s) & (e_w == w)
                                      & (e_k >= K[s, w]))[0]
                    assert len(ssel) <= nb * ST
                    boff = spb_off[(g, w)] + bloc
                    for i, ei in enumerate(ssel):
                        b, p = divmod(i, ST)
                        idx_flat[(rs + b) * ST + p] = e_srow[ei] - w * WIN
                        dloc[p, boff + b] = np.float16(e_slot[ei])
                ncols = r_gw * (ST // 16)
                wrapped = idx_flat.astype(np.int16).reshape(ncols, 16).T
                c0 = col_off[(g, w)]
                eidx[:, c0:c0 + ncols] = np.tile(wrapped, (8, 1))
        plans.append(dict(eidx=eidx, dstloc=dloc, pos_of=pos_of))

    slots_round = int(rows_gw.sum()) * ST
    meta = dict(K=K, SPB=SPB, groups=groups, rows_gw=rows_gw,
                grid_layout=grid_layout, spill_layout=spill_layout,
                spb_off=spb_off, col_off=col_off, TOTCOLS=TOTCOLS,
                NSPB=max(NSPB, 1), NBMAX=NBMAX, ROWS_CAP=ROWS_CAP, zpad=zpad,
                slots_round=slots_round, pos_of=pos_of)
    return plans, meta


def build_program(cfg, meta, sim_mode=False):
    NCORE, SHARD, NPAD = cfg["NCORE"], cfg["SHARD"], cfg["NPAD"]
    NWIN, WIN, NST, SB, NG, F, H = (cfg[k] for k in
                                    ("NWIN", "WIN", "NST", "SB", "NG", "F", "H"))
    groups = meta["groups"]
    rows_gw = meta["rows_gw"]
    grid_layout = meta["grid_layout"]
    spill_layout = meta["spill_layout"]
    spb_off = meta["spb_off"]
    col_off = meta["col_off"]
    TOTCOLS, NSPB, ROWS_CAP = meta["TOTCOLS"], meta["NSPB"], meta["ROWS_CAP"]
    NBMAX = meta["NBMAX"]

    TDT = F8 if os.environ.get("KDT", "f8") == "f8" else F16
    TB = H                               # payload elements per table row
    TROW = 256 if TDT == F8 else 128     # stored elements per 256B row

    nc = Bacc(trn_type="TRN2", num_devices=NCORE)

    feat_tt = nc.dram_tensor("feat_tt", [F, SHARD], F16, kind="ExternalInput")
    nsrc = nc.dram_tensor("nsrc", [128, NST], F32, kind="ExternalInput")
    nprod = nc.dram_tensor("nprod", [128, NST], F32, kind="ExternalInput")
    ndst = nc.dram_tensor("ndst", [128, NST], F32, kind="ExternalInput")
    w1_16 = nc.dram_tensor("w1_16", [F, H], F16, kind="ExternalInput")
    wmu_16 = nc.dram_tensor("wmu_16", [H, H], F16, kind="ExternalInput")
    wsig_16 = nc.dram_tensor("wsig_16", [H, H], F16, kind="ExternalInput")
    b1_rep = nc.dram_tensor("b1_rep", [128, H], F32, kind="ExternalInput")
    bmu_col = nc.dram_tensor("bmu_col", [H, 1], F32, kind="ExternalInput")
    bsig_col = nc.dram_tensor("bsig_col", [H, 1], F32, kind="ExternalInput")
    eye16_d = nc.dram_tensor("eye16", [128, 128], F16, kind="ExternalInput")
    ident_d = nc.dram_tensor("ident_t", [128, 128], TDT, kind="ExternalInput")
    iota_rep_d = nc.dram_tensor("iota_rep", [128, 128, NBMAX], F16,
                                kind="ExternalInput")
    eidx_d = nc.dram_tensor("eidx", [128, TOTCOLS], I16, kind="ExternalInput")
    dstloc_d = nc.dram_tensor("dstloc", [128, NSPB], F16, kind="ExternalInput")
    noise_t = nc.dram_tensor("noise_t", [H, SHARD], F16, kind="ExternalInput")
    z_out = nc.dram_tensor("z_out", [H, SHARD], F16, kind="ExternalOutput")
    dbg = bool(int(os.environ.get("KDBG", "0")))
    if dbg:
        x1_dbg = nc.dram_tensor("x1_dbg", [128, NST, TROW], TDT,
                                kind="ExternalOutput")
        h_dbg = nc.dram_tensor("h_dbg", [128, NST, TROW], TDT,
                               kind="ExternalOutput")

    x1_shard = nc.dram_tensor("x1_shard", [128, NST, TROW], TDT, kind="Internal")
    h_shard = nc.dram_tensor("h_shard", [128, NST, TROW], TDT, kind="Internal")
    x1_table = nc.dram_tensor("x1_table", [NPAD, TROW], TDT, kind="Internal",
                              addr_space="Shared")
    h_table = nc.dram_tensor("h_table", [NPAD, TROW], TDT, kind="Internal",
                             addr_space="Shared")
    cgroups = [list(range(NCORE))]

    spill_oh_dt = F16 if os.environ.get("KMIX", "1") == "1" else TDT

    with tile.TileContext(nc) as tc, ExitStack() as ctx:
        consts = ctx.enter_context(tc.tile_pool(name="consts", bufs=1))

        def cload(dram, shape, dtype, tag):
            t = consts.tile(shape, dtype, tag=tag)
            nc.sync.dma_start(t[:], dram[:])
            return t

        w1_sb = cload(w1_16, [F, H], F16, "w1")
        nsrc_sb = cload(nsrc, [128, NST], F32, "nsrc")

        # ---------------- phase 1: x1 = (feat*nsrc) @ W1 on own shard -------
        PG = 14  # physical supertiles per feat tile
        with tc.tile_pool(name="p1", bufs=3) as p1, \
             tc.tile_pool(name="p1ps", bufs=8, space="PSUM") as p1ps:
            for g0 in range(NST // PG):
                ftg = p1.tile([F, PG * 128], F16, tag="ftg")
                nc.sync.dma_start(ftg[:],
                                  feat_tt[:, g0 * PG * 128:(g0 + 1) * PG * 128])
                xg = p1.tile([128, PG, H], TDT, tag="xg")
                for si in range(PG):
                    s = g0 * PG + si
                    x1p = p1ps.tile([128, H], F32, tag="x1p")
                    nc.tensor.matmul(x1p[:], ftg[:, si * 128:(si + 1) * 128],
                                     w1_sb[:], start=True, stop=True)
                    nc.vector.tensor_scalar(xg[:, si, :], x1p[:],
                                            nsrc_sb[:, s:s + 1], None, ALU.mult)
                nc.sync.dma_start(
                    x1_shard[:, g0 * PG:(g0 + 1) * PG, 0:H], xg[:])
                if sim_mode:
                    # collective stand-in, pipelined per group
                    nc.sync.dma_start(
                        x1_table[0:SHARD, :]
                        .rearrange("(p s) e -> p s e", p=128)
                        [:, g0 * PG:(g0 + 1) * PG, :],
                        x1_shard[:, g0 * PG:(g0 + 1) * PG, :])

        # round-only consts: loaded after phase-1 emission
        wmu_sb = cload(wmu_16, [H, H], F16, "wmu")
        wsig_sb = cload(wsig_16, [H, H], F16, "wsig")
        ndst_sb = cload(ndst, [128, NST], F32, "ndst")
        nprod_sb = cload(nprod, [128, NST], F32, "nprod")
        b1_sb = cload(b1_rep, [128, H], F32, "b1")
        bmu_sb = cload(bmu_col, [H, 1], F32, "bmu")
        bsig_sb = cload(bsig_col, [H, 1], F32, "bsig")
        eye16 = cload(eye16_d, [128, 128], F16, "eye16")
        ident = cload(ident_d, [128, 128], TDT, "ident")
        iota_rep = cload(iota_rep_d, [128, 128, NBMAX], F16, "iota_rep")
        dstloc_sb = cload(dstloc_d, [128, NSPB], F16, "dstloc")
        eidx_sb = cload(eidx_d, [128, TOTCOLS], I16, "eidx")

        if dbg:
            nc.sync.dma_start(x1_dbg[:], x1_shard[:, :, :])
        if not sim_mode:
            nc.gpsimd.collective_compute("AllGather", ALU.bypass, cgroups,
                                         ins=[x1_shard[:]], outs=[x1_table[:]])

        HB = min(SB, 7)  # supertiles per PSUM bank

        # ---------------- message-passing round ----------------------------
        def mp_round(table, epilogue, pre_round, post_group, rtag):
            with tc.tile_pool(name=f"msgs{rtag}", bufs=3) as msgs, \
                 tc.tile_pool(name=f"ohp{rtag}", bufs=2) as ohp, \
                 tc.tile_pool(name=f"aggps{rtag}", bufs=2, space="PSUM") as aggps, \
                 tc.tile_pool(name=f"epi{rtag}", bufs=4) as epi, \
                 tc.tile_pool(name=f"episb{rtag}", bufs=4) as episb, \
                 tc.tile_pool(name=f"stg{rtag}", bufs=1) as stg, \
                 tc.tile_pool(name=f"epips{rtag}", bufs=2, space="PSUM") as epips, \
                 tc.tile_pool(name=f"epips2{rtag}", bufs=2, space="PSUM") as epips2:
                rctx = pre_round(stg)

                def run_epilogue(g, banks):
                    for j, s in enumerate(groups[g]):
                        agg = banks[j // HB][:, (j % HB) * H:(j % HB + 1) * H]
                        epilogue(g, j, s, agg, rctx, epi, episb, epips, epips2)
                    post_group(g, rctx)

                pending = None
                for g in range(NG):
                    sts = groups[g]
                    # emit window 0 last (its table rows overlap the own-shard
                    # copy; avoids Pool head-of-line blocking at the boundary)
                    mt = {}
                    for w in (1, 2, 3, 0):
                        r_gw = int(rows_gw[g, w])
                        if r_gw == 0:
                            continue
                        m = msgs.tile([128, ROWS_CAP, TB], TDT, tag=f"m{w}")
                        raw_gather(
                            nc.gpsimd, m[:, 0:r_gw, :],
                            table[w * WIN:(w + 1) * WIN, 0:TB],
                            eidx_sb[:, col_off[(g, w)]:
                                    col_off[(g, w)] + r_gw * (ST // 16)],
                            num_idxs=r_gw * ST, num_idxs_reg=r_gw * ST,
                            elem_size=TB, elem_step=TROW)
                        mt[w] = m
                    # spill one-hots (DVE; independent of gathers)
                    ohs = {}
                    for w in range(NWIN):
                        nb = sum(n for (_, _, _, n, _) in spill_layout[(g, w)])
                        if nb == 0:
                            continue
                        b0 = spb_off[(g, w)]
                        oh = ohp.tile([128, 128, NBMAX], spill_oh_dt,
                                      tag=f"oh{w}")
                        nc.vector.tensor_tensor(
                            oh[:, :, 0:nb], iota_rep[:, :, 0:nb],
                            dstloc_sb[:, None, b0:b0 + nb]
                            .broadcast_to([128, 128, nb]),
                            ALU.is_equal)
                        ohs[w] = oh
                    # PSUM banks: HB supertiles each
                    nbank = -(-SB // HB)
                    banks = []
                    for hb in range(nbank):
                        nsi = min(HB, SB - hb * HB)
                        ab = aggps.tile([128, nsi * H], F32, tag=f"ab{hb}")
                        nc.vector.memset(ab[:], 0.0)
                        banks.append(ab)
                    # matmul op list; last op per bank carries the stop flag
                    ops = []
                    for w in range(NWIN):
                        for (k, nk, rs) in grid_layout[(g, w)]:
                            for hb in range(nbank):
                                lo, hi = hb * HB, min((hb + 1) * HB, nk)
                                if lo >= nk:
                                    break
                                ops.append(("g", w, hb, lo, hi, rs))
                        for (j, s, rs, nb, bloc) in spill_layout[(g, w)]:
                            for b in range(nb):
                                ops.append(("s", w, j // HB, j, rs + b,
                                            bloc + b))
                    last_of = {}
                    for i, op in enumerate(ops):
                        last_of[op[2]] = i
                    for i, op in enumerate(ops):
                        stop = (last_of[op[2]] == i)
                        if op[0] == "g":
                            _, w, hb, lo, hi, rs = op
                            nc.tensor.matmul(
                                banks[hb][:, 0:(hi - lo) * H], ident[:],
                                mt[w][:, rs + lo:rs + hi, 0:H],
                                start=False, stop=stop)
                        else:
                            _, w, hb, j, row, bi = op
                            jj = j % HB
                            nc.tensor.matmul(
                                banks[hb][:, jj * H:(jj + 1) * H],
                                ohs[w][:, :, bi],
                                mt[w][:, row, 0:H],
                                start=False, stop=stop)
                    if pending is not None:
                        run_epilogue(*pending)
                    pending = (g, banks)
                if pending is not None:
                    run_epilogue(*pending)

        # ---------------- round 1: h ----------------------------------------
        def pre_r1(stg):
            hg = stg.tile([128, NST, H], TDT, tag="hg")
            return dict(hg=hg)

        def post_g1(g, rctx):
            nc.sync.dma_start(h_shard[:, g * SB:(g + 1) * SB, 0:H],
                              rctx["hg"][:, g * SB:(g + 1) * SB, :])
            if sim_mode:
                nc.sync.dma_start(
                    h_table[0:SHARD, :]
                    .rearrange("(p s) e -> p s e", p=128)
                    [:, g * SB:(g + 1) * SB, :],
                    h_shard[:, g * SB:(g + 1) * SB, :])

        def epi_round1(g, j, s, agg, rctx, epi, episb, epips, epips2):
            if cfg.get("B1Z", True):
                nc.scalar.activation(rctx["hg"][:, s, :], agg, ACTF.Relu,
                                     scale=nprod_sb[:, s:s + 1])
            else:
                hp = epi.tile([128, H], F32, tag="hp")
                nc.vector.scalar_tensor_tensor(hp[:], agg, ndst_sb[:, s:s + 1],
                                               b1_sb[:], ALU.mult, ALU.add)
                nc.scalar.activation(rctx["hg"][:, s, :], hp[:], ACTF.Relu,
                                     scale=nsrc_sb[:, s:s + 1])

        # ---------------- round 2: z -----------------------------------------
        def pre_r2(stg):
            zg = stg.tile([H, SHARD], F16, tag="zg")
            ng = stg.tile([H, SHARD], F16, tag="ng")
            nc.sync.dma_start(ng[:], noise_t[:])
            return dict(zg=zg, ng=ng)

        def post_g2(g, rctx):
            nc.sync.dma_start(z_out[:, g * SB * 128:(g + 1) * SB * 128],
                              rctx["zg"][:, g * SB * 128:(g + 1) * SB * 128])

        bz = cfg.get("BZ", True)  # b_mu == b_sig == 0 fast path

        def epi_round2(g, j, s, agg, rctx, epi, episb, epips, epips2):
            a2s = epi.tile([128, H], F16, tag="a2s")
            nc.scalar.activation(a2s[:], agg, ACTF.Identity,
                                 scale=ndst_sb[:, s:s + 1])
            a2tp = epips.tile([H, 128], F16, tag="a2tp")
            nc.tensor.matmul(a2tp[:], a2s[:], eye16[:], is_transpose=True)
            a2t = epi.tile([H, 128], F16, tag="a2t")
            nc.vector.tensor_scalar(a2t[:], a2tp[:], 1.0, None, ALU.mult)
            musg = epips2.tile([H, 2, 128], F32, tag="musg")
            nc.tensor.matmul(musg[:, 0, :], wmu_sb[:], a2t[:], start=True,
                             stop=True)
            nc.tensor.matmul(musg[:, 1, :], wsig_sb[:], a2t[:], start=True,
                             stop=True)
            es = episb.tile([H, 128], F32, tag="es")
            if bz:
                nc.scalar.activation(es[:], musg[:, 1, :], ACTF.Exp)
            else:
                nc.scalar.activation(es[:], musg[:, 1, :], ACTF.Exp,
                                     bias=bsig_sb[:])
            nz = episb.tile([H, 128], F32, tag="nz")
            nc.vector.scalar_tensor_tensor(
                nz[:], rctx["ng"][:, s * 128:(s + 1) * 128], 1.0, es[:],
                ALU.mult, ALU.mult)
            if bz:
                nc.vector.scalar_tensor_tensor(
                    rctx["zg"][:, s * 128:(s + 1) * 128], musg[:, 0, :], 0.0,
                    nz[:], ALU.add, ALU.add)
            else:
                mub = episb.tile([H, 128], F32, tag="mub")
                nc.scalar.activation(mub[:], musg[:, 0, :], ACTF.Identity,
                                     bias=bmu_sb[:])
                nc.vector.scalar_tensor_tensor(
                    rctx["zg"][:, s * 128:(s + 1) * 128], mub[:], 0.0, nz[:],
                    ALU.add, ALU.add)

        kphase = int(os.environ.get("KPHASE", "4"))
        if kphase >= 2:
            mp_round(x1_table, epi_round1, pre_r1, post_g1, "a")
        if kphase >= 3:
            if dbg:
                nc.sync.dma_start(h_dbg[:], h_shard[:, :, :])
            if not sim_mode:
                nc.gpsimd.collective_compute("AllGather", ALU.bypass, cgroups,
                                             ins=[h_shard[:]],
                                             outs=[h_table[:]])
        if kphase >= 4:
            mp_round(h_table, epi_round2, pre_r2, post_g2, "b")

    nc.finalize()
    return nc


def host_inputs(feat, src, dst, noise, W1, b1, W_mu, b_mu, W_sig, b_sig,
                cfg, plans, meta):
    N, NCORE, SHARD, NPAD = (cfg[k] for k in ("N", "NCORE", "SHARD", "NPAD"))
    NST, F, H, NWIN = cfg["NST"], cfg["F"], cfg["H"], cfg["NWIN"]
    NSPB = meta["NSPB"]
    pos_of = meta["pos_of"]
    feat = np.asarray(feat, dtype=np.float32)
    noise = np.asarray(noise, dtype=np.float32)
    src = np.asarray(src); dst = np.asarray(dst)

    deg_out = np.bincount(src, minlength=NPAD).astype(np.float32)
    deg_in = np.bincount(dst, minlength=NPAD).astype(np.float32)
    norm_src = np.maximum(deg_out, 1.0) ** -0.5
    norm_dst = np.maximum(deg_in, 1.0) ** -0.5

    inv = np.empty(NPAD, dtype=np.int64)
    inv[pos_of] = np.arange(NPAD)          # node at each position

    featp = np.zeros((NPAD, F), dtype=np.float32)
    featp[pos_of[:N]] = feat
    noisep = np.zeros((NPAD, H), dtype=np.float32)
    noisep[pos_of[:N]] = noise
    ns_p = norm_src[inv]
    nd_p = norm_dst[inv]

    TDT8 = os.environ.get("KDT", "f8") == "f8"
    eye16 = np.eye(128, dtype=np.float16)
    if TDT8:
        import ml_dtypes
        ident = np.eye(128).astype(ml_dtypes.float8_e4m3fn)
    else:
        ident = np.eye(128, dtype=np.float16)
    iota_rep = np.tile(np.arange(128, dtype=np.float16)[None, :, None],
                       (128, 1, meta["NBMAX"]))
    shared = dict(
        w1_16=np.asarray(W1, dtype=np.float16),
        wmu_16=np.asarray(W_mu, dtype=np.float16),
        wsig_16=np.asarray(W_sig, dtype=np.float16),
        b1_rep=np.tile(np.asarray(b1, dtype=np.float32)[None, :], (128, 1)),
        bmu_col=np.asarray(b_mu, dtype=np.float32).reshape(H, 1),
        bsig_col=np.asarray(b_sig, dtype=np.float32).reshape(H, 1),
        eye16=eye16, ident_t=ident, iota_rep=iota_rep,
    )
    in_maps = []
    for c in range(NCORE):
        lo, hi = c * SHARD, (c + 1) * SHARD
        m = dict(shared)
        m["feat_tt"] = featp[lo:hi].T.astype(np.float16).copy()
        m["nsrc"] = ns_p[lo:hi].reshape(NST, 128).T.copy()
        m["nprod"] = (ns_p * nd_p)[lo:hi].reshape(NST, 128).T.copy()
        m["ndst"] = nd_p[lo:hi].reshape(NST, 128).T.copy()
        m["noise_t"] = noisep[lo:hi].T.astype(np.float16).copy()
        m["eidx"] = plans[c]["eidx"]
        m["dstloc"] = plans[c]["dstloc"].astype(np.float16)
        in_maps.append(m)
    return in_maps


def run(feat, src, dst, noise, W1, b1, W_mu, b_mu, W_sig, b_sig,
        cfg=None, **spmd_kwargs):
    if cfg is None:
        cfg = default_cfg(feat.shape[0], src.shape[0], feat.shape[1],
                          W1.shape[1])
    cfg["B1Z"] = bool(np.all(np.asarray(b1) == 0.0))
    cfg["BZ"] = bool(np.all(np.asarray(b_mu) == 0.0)
                     and np.all(np.asarray(b_sig) == 0.0))
    plans, meta = build_plan(src, dst, cfg)
    nc = build_program(cfg, meta)
    in_maps = host_inputs(feat, src, dst, noise, W1, b1, W_mu, b_mu,
                          W_sig, b_sig, cfg, plans, meta)
    import time as _time
    last_exc = None
    for attempt in range(3):
        try:
            res = run_bass_kernel_spmd(nc, in_maps,
                                       core_ids=list(range(cfg["NCORE"])),
                                       **spmd_kwargs)
            break
        except Exception as e:
            last_exc = e
            _time.sleep(10.0)
    else:
        raise last_exc
    zs = [r["z_out"].T for r in res.results]          # [SHARD, H] each
    z_pos = np.concatenate(zs, axis=0)                # position-major
    z = z_pos[meta["pos_of"][:cfg["N"]]]
    return z.astype(np.float32), res


def kernel(feat, src, dst, noise, W1, b1, W_mu, b_mu, W_sig, b_sig):
    z, _ = run(feat, src, dst, noise, W1, b1, W_mu, b_mu, W_sig, b_sig)
    return z


# revision 38
# speedup vs baseline: 1.3740x; 1.0452x over previous
"""Trainium2 Bass kernel for nn_Encoder (VGAE-style GNN encoder).

Computation (see reference):
  deg/norms from src/dst; h = relu(ndst * segsum_dst((feat*nsrc @ W1)[src]))
  agg2 = segsum_dst((h*nsrc)[src]); mu = (agg2*ndst) @ W_mu + b_mu ; ls likewise
  z = mu + noise * exp(log_sigma)

Strategy (graph/data parallel, dst-sharded, K-grid + one-hot-spill reduce):
  - Nodes globally sorted by in-degree and striped across (core, supertile)
    so every core sees the same degree profile (the SPMD program is shared
    across cores, so all gather/matmul shapes are max-over-cores). Host
    pre-permutes feat/noise/norms into table order, un-permutes z. Fake (pad)
    nodes are spread over the last supertile of every core so each gather
    window contains a zero row for pad slots.
  - Message tables (x1, h) stored fp8e4 in 256B-stride rows (64B payload):
    each dma_gather descriptor hits the 7ns cost-model floor (vs 11.4ns for
    128B f16). 4 source windows (int16 gather index range).
  - Per (supertile, window): a K-deep grid where slot (k, p) holds the k-th
    window-w in-edge of dst slot p (pad slots fetch the window's zero row).
    The reduce is a PSUM-accumulated matmul with an fp8 identity stationary,
    batched over supertile prefixes per (window, k, bank) -- K is monotone
    along each group so prefixes are contiguous -- with no per-edge DVE
    one-hot work. Edges beyond the grid depth take a spill path with classic
    one-hot blocks (f16 stationary x fp8 moving, mixed-dtype matmul). K is
    host-optimized per (supertile, window); serpentine supertile->group
    dealing equalizes per-group rows (SBUF tile caps).
  - Software-pipelined emission: group g's epilogue is issued between group
    g+1's gathers and its reduce matmuls, so the in-order PE queue drains
    epilogues during gather waits. Per-group h/z stores; in sim mode the
    AllGathers are stood in by per-group strided local copies.
  - Epilogues: relu on ACT with nprod = nsrc*ndst (folds round-1 post-scale
    and round-2 pre-scale; b1==0); round 2 splits work across ACT (ndst
    scale-copy, exp) / PE (transpose, W_mu, W_sig) / DVE (PSUM->SBUF copy,
    z = mu + noise*exp(ls) with mu read straight from PSUM since b_mu==0).
"""

import os
import sys
import numpy as np
from contextlib import ExitStack

if "/opt/trn_rl_repo" not in sys.path:
    sys.path.insert(0, "/opt/trn_rl_repo")

import concourse.bass as bass
import concourse.mybir as mybir
import concourse.tile as tile
from concourse.bacc import Bacc
from concourse.bass_utils import run_bass_kernel_spmd

F16 = mybir.dt.float16
F32 = mybir.dt.float32
F8 = mybir.dt.float8e4
I16 = mybir.dt.int16
ALU = mybir.AluOpType
ACTF = mybir.ActivationFunctionType

ST = 128


def raw_gather(gp, out_ap, in_ap, idxs_ap, num_idxs, num_idxs_reg, elem_size,
               elem_step, single_packet=False, queue_num=0):
    """dma_gather without the elem_size_bytes%256 assert (non-transpose, DRAM
    source, 256B-aligned row stride): fetches sub-row payloads (64B fp8 /
    128B f16) from a 256B-stride table."""
    from concourse.ap_utils import ap_is_contiguous
    assert idxs_ap.dtype == mybir.dt.int16
    assert in_ap.dtype == out_ap.dtype
    assert ap_is_contiguous(in_ap.ap[1:])
    assert ap_is_contiguous(out_ap.ap[1:])
    assert ap_is_contiguous(idxs_ap.ap[1:])
    assert in_ap.ap[-1][1] == out_ap.ap[-1][1] == elem_size
    assert out_ap.ap[0][1] * out_ap.ap[1][1] == -(-num_idxs // 128) * 128
    assert in_ap.ap[0][0] == elem_step
    stride_bytes = elem_step * mybir.dt.size(in_ap.dtype)
    stride_bytes_256 = stride_bytes // 256
    assert stride_bytes % 256 == 0 and stride_bytes_256 < 256
    _in_ap = gp.lower_ap_dma(in_ap, for_custom_bir_dma=True)
    _idxs_ap = gp.lower_ap(idxs_ap)
    _out_ap = gp.lower_ap(out_ap)
    return gp.add_instruction(
        mybir.InstDMAGatherAnt(
            name=gp.bass.get_next_instruction_name(),
            ins=[*_in_ap, _idxs_ap, gp.lower_val_access(gp.to_reg(num_idxs_reg))],
            outs=[_out_ap],
            transpose=False,
            num_idxs=num_idxs,
            elem_size=elem_size,
            stride_bytes_256=stride_bytes_256,
            gen_mode=0,
            single_packet=single_packet,
            queue_num=queue_num,
            sbuf_tokens_per_rank=0,
            sbuf_free_dim_per_rank=0,
            sbuf_free_dim_pad_per_rank=0,
            sbuf_byte_offset=0,
        )
    )


def default_cfg(n, e, f, h):
    ncore = 8
    shard = -(-n // (ncore * ST)) * ST
    npad = shard * ncore
    nst = shard // ST
    nwin = 4
    win = -(-npad // nwin)  # p-major table rows per window
    assert win <= 32768, "int16 gather index range"
    sb = int(os.environ.get("KSB", "14"))
    while nst % sb:
        sb -= 1
    return dict(N=n, E=e, F=f, H=h, NCORE=ncore, SHARD=shard, NPAD=npad,
                NWIN=nwin, WIN=win, NST=nst, SB=sb, NG=nst // sb)


def _serpentine_deal(nst, ng, sb):
    """deal_to_st[d] = table supertile index (g*sb + j) of the d-th
    degree-ranked supertile: snake-deal ranks across groups so per-group
    row totals stay balanced while j stays degree-descending in each group."""
    deal_to_st = np.empty(nst, dtype=np.int64)
    d = 0
    for p in range(sb):
        order = range(ng) if p % 2 == 0 else range(ng - 1, -1, -1)
        for gi in order:
            deal_to_st[d] = gi * sb + p
            d += 1
    return deal_to_st


def build_plan(src, dst, cfg):
    """Host-side index preprocessing. Returns per-core arrays + uniform meta."""
    N, NCORE = cfg["N"], cfg["NCORE"]
    SHARD, NPAD, NWIN, WIN, NST, SB, NG = (
        cfg[k] for k in ("SHARD", "NPAD", "NWIN", "WIN", "NST", "SB", "NG"))
    src = np.asarray(src).astype(np.int64)
    dst = np.asarray(dst).astype(np.int64)

    deg_in = np.bincount(dst, minlength=NPAD)
    # global in-degree sort; stripe supertiles across cores so all cores share
    # one degree profile (SPMD program uniformity)
    order = np.argsort(-deg_in, kind="stable")
    # spread the fake (zero) nodes across the last 8 global supertiles (one
    # per core) so every gather window contains a zero row for pad slots
    nfake = NPAD - N
    if nfake:
        tailn = max(1024, -(-nfake // 8) * 8 * 2)
        tailn = min(tailn, NPAD)
        tail = order[-tailn:].copy()
        fk = tail[tail >= N]
        rl = tail[tail < N]
        newtail = np.empty_like(tail)
        taken = np.zeros(tailn, dtype=bool)
        nst_tail = min(8, tailn // ST)
        for j, f in enumerate(fk):
            o2 = (j % nst_tail) * ST + j // nst_tail
            newtail[o2] = f
            taken[o2] = True
        newtail[~taken] = rl
        order[-tailn:] = newtail
    rank = np.arange(NPAD)
    sti = rank // ST                      # global sorted supertile 0..NST*8-1
    st_core = sti % NCORE
    deal_to_st = _serpentine_deal(NST, NG, SB)
    st_in_core = deal_to_st[sti // NCORE]
    posn = st_core * SHARD + st_in_core * ST + (rank % ST)
    pos_of = np.empty(NPAD, dtype=np.int64)
    pos_of[order] = posn

    # p-major table row of a position (matches phase-1 store layout)
    def srow_of(pos):
        return (pos // SHARD) * SHARD + (pos % SHARD) % ST * NST + (pos % SHARD) // ST

    srow_all = srow_of(np.arange(NPAD))
    # zero-pad target row per window: a fake position (node id >= N) in range
    fake_pos = pos_of[N:] if NPAD > N else None
    zpad = np.zeros(NWIN, dtype=np.int64)
    if fake_pos is not None and len(fake_pos):
        fr = np.sort(srow_of(fake_pos))
        for w in range(NWIN):
            cand = fr[(fr >= w * WIN) & (fr < (w + 1) * WIN)]
            assert len(cand), f"no fake row in window {w}"
            zpad[w] = cand[0]
    groups = [[g * SB + j for j in range(SB)] for g in range(NG)]

    dstp = pos_of[dst]
    srcp = pos_of[src]
    srow = srow_of(srcp)
    wofe = srow // WIN
    core = dstp // SHARD
    stl = (dstp % SHARD) // ST
    slot = dstp % ST

    # per-core per-(st,w) degree per slot
    deg = np.zeros((NCORE, NST, NWIN, ST), dtype=np.int32)
    key = ((core * NST + stl) * NWIN + wofe) * ST + slot
    cnts = np.bincount(key, minlength=NCORE * NST * NWIN * ST)
    deg = cnts.reshape(NCORE, NST, NWIN, ST)

    # K optimization per (st, w), SPMD-uniform across cores:
    # cost(K) = 128*K + 128*max_c ceil(spill_c(K)/128)
    degM = deg  # [C, NST, NWIN, ST]
    maxdeg = int(degM.max())
    # a spill block costs more than a grid row (DVE one-hot build + per-block
    # Ldweights/matmul issue vs one batched identity matmul)
    spill_w = float(os.environ.get("KSPW", "1.2"))
    K = np.zeros((NST, NWIN), dtype=np.int32)
    SPB = np.zeros((NST, NWIN), dtype=np.int32)
    for s in range(NST):
        for w in range(NWIN):
            d = degM[:, s, w, :]  # [C, ST]
            best, bestk, bestspb = None, 0, 0
            for k in range(0, min(maxdeg, int(d.max())) + 1):
                spill = np.maximum(d - k, 0).sum(axis=1).max()
                spb = -(-int(spill) // ST)
                c = ST * k + spill_w * ST * spb
                if best is None or c < best:
                    best, bestk, bestspb = c, k, spb
            K[s, w], SPB[s, w] = bestk, bestspb
    # monotone non-increasing K along each group's st order (prefix matmuls)
    for g in range(NG):
        sts = groups[g]
        for w in range(NWIN):
            for j in range(SB - 2, -1, -1):
                K[sts[j], w] = max(K[sts[j], w], K[sts[j + 1], w])
    # recompute spill block counts at the final K
    for s in range(NST):
        for w in range(NWIN):
            spill = np.maximum(degM[:, s, w, :] - K[s, w], 0).sum(axis=1).max()
            SPB[s, w] = -(-int(spill) // ST)

    # uniform row layout per (g, w): grid rows k-major, then spill rows
    rows_gw = np.zeros((NG, NWIN), dtype=np.int64)
    grid_layout = {}   # (g,w) -> list of (k, n_k, row_start)
    spill_layout = {}  # (g,w) -> list of (j, st, row_start, nblocks)
    for g in range(NG):
        sts = groups[g]
        for w in range(NWIN):
            r = 0
            gl = []
            kmax = int(K[sts[0], w])
            for k in range(kmax):
                nk = int(sum(1 for s in sts if K[s, w] > k))
                gl.append((k, nk, r))
                r += nk
            sl = []
            bloc = 0
            for j, s in enumerate(sts):
                nb = int(SPB[s, w])
                if nb:
                    sl.append((j, s, r, nb, bloc))
                    r += nb
                    bloc += nb
            grid_layout[(g, w)] = gl
            spill_layout[(g, w)] = sl
            rows_gw[g, w] = r
    ROWS_CAP = int(rows_gw.max())
    # spill block base offsets in dstloc, ordered (g, w); NBMAX = per-(g,w) max
    spb_off = {}
    acc = 0
    NBMAX = 1
    for g in range(NG):
        for w in range(NWIN):
            spb_off[(g, w)] = acc
            nb_gw = sum(nb for (_, _, _, nb, _) in spill_layout[(g, w)])
            NBMAX = max(NBMAX, nb_gw)
            acc += nb_gw
    NSPB = acc
    # eidx col offsets per (g, w)
    col_off = {}
    acc = 0
    for g in range(NG):
        for w in range(NWIN):
            col_off[(g, w)] = acc
            acc += int(rows_gw[g, w]) * (ST // 16)
    TOTCOLS = acc

    # per-core edge placement
    plans = []
    for c in range(NCORE):
        sel = core == c
        e_st, e_w, e_slot = stl[sel], wofe[sel], slot[sel]
        e_srow = srow[sel]
        o = np.lexsort((e_srow, e_slot, e_w, e_st))
        e_st, e_w, e_slot, e_srow = e_st[o], e_w[o], e_slot[o], e_srow[o]
        # rank of edge within its (st, w, slot) list
        key2 = (e_st * NWIN + e_w) * ST + e_slot
        # edges sorted by key2 groups (lexsort above ensures grouping)
        o2 = np.argsort(key2, kind="stable")
        k2s = key2[o2]
        within = np.arange(len(k2s)) - np.searchsorted(k2s, k2s, side="left")
        e_k = np.empty(len(k2s), dtype=np.int64)
        e_k[o2] = within

        eidx = np.zeros((128, TOTCOLS), dtype=np.int16)
        dloc = np.full((128, max(NSPB, 1)), 300.0, dtype=np.float16)
        st_j = {}
        for g in range(NG):
            for j, s in enumerate(groups[g]):
                st_j[s] = (g, j)
        # grid placement
        for g in range(NG):
            sts = groups[g]
            jmap = -np.ones(NST, dtype=np.int64)
            for j, s in enumerate(sts):
                jmap[s] = j
            for w in range(NWIN):
                r_gw = int(rows_gw[g, w])
                if r_gw == 0:
                    continue
                idx_flat = np.full(r_gw * ST, zpad[w] - w * WIN, dtype=np.int64)
                # grid rows
                gsel = (np.isin(e_st, sts) & (e_w == w)
                        & (e_k < K[e_st, w]))
                gs = np.nonzero(gsel)[0]
                if len(gs):
                    kk = e_k[gs]
                    ss = e_st[gs]
                    jj = jmap[ss]
                    # row of (k, st j): row_start(k) + position of j among
                    # sts with K> k (prefix since K monotone in j)
                    gl = grid_layout[(g, w)]
                    rstart = np.zeros(int(K[sts[0], w]) + 1, dtype=np.int64)
                    for (k, nk, rs) in gl:
                        rstart[k] = rs
                    rows = rstart[kk] + jj
                    idx_flat[rows * ST + e_slot[gs]] = e_srow[gs] - w * WIN
                # spill rows
                for (j, s, rs, nb, bloc) in spill_layout[(g, w)]:
                    ssel = np.nonzero((e_st == s) & (e_w == w)
                                      & (e_k >= K[s, w]))[0]
                    assert len(ssel) <= nb * ST
                    boff = spb_off[(g, w)] + bloc
                    for i, ei in enumerate(ssel):
                        b, p = divmod(i, ST)
                        idx_flat[(rs + b) * ST + p] = e_srow[ei] - w * WIN
                        dloc[p, boff + b] = np.float16(e_slot[ei])
                ncols = r_gw * (ST // 16)
                wrapped = idx_flat.astype(np.int16).reshape(ncols, 16).T
                c0 = col_off[(g, w)]
                eidx[:, c0:c0 + ncols] = np.tile(wrapped, (8, 1))
        plans.append(dict(eidx=eidx, dstloc=dloc, pos_of=pos_of))

    slots_round = int(rows_gw.sum()) * ST
    meta = dict(K=K, SPB=SPB, groups=groups, rows_gw=rows_gw,
                grid_layout=grid_layout, spill_layout=spill_layout,
                spb_off=spb_off, col_off=col_off, TOTCOLS=TOTCOLS,
                NSPB=max(NSPB, 1), NBMAX=NBMAX, ROWS_CAP=ROWS_CAP, zpad=zpad,
                slots_round=slots_round, pos_of=pos_of)
    return plans, meta


def build_program(cfg, meta, sim_mode=False):
    NCORE, SHARD, NPAD = cfg["NCORE"], cfg["SHARD"], cfg["NPAD"]
    NWIN, WIN, NST, SB, NG, F, H = (cfg[k] for k in
                                    ("NWIN", "WIN", "NST", "SB", "NG", "F", "H"))
    groups = meta["groups"]
    rows_gw = meta["rows_gw"]
    grid_layout = meta["grid_layout"]
    spill_layout = meta["spill_layout"]
    spb_off = meta["spb_off"]
    col_off = meta["col_off"]
    TOTCOLS, NSPB, ROWS_CAP = meta["TOTCOLS"], meta["NSPB"], meta["ROWS_CAP"]
    NBMAX = meta["NBMAX"]

    TDT = F8 if os.environ.get("KDT", "f8") == "f8" else F16
    TB = H                               # payload elements per table row
    TROW = 256 if TDT == F8 else 128     # stored elements per 256B row

    nc = Bacc(trn_type="TRN2", num_devices=NCORE)

    feat_tt = nc.dram_tensor("feat_tt", [F, SHARD], F16, kind="ExternalInput")
    nsrc = nc.dram_tensor("nsrc", [128, NST], F32, kind="ExternalInput")
    nprod = nc.dram_tensor("nprod", [128, NST], F32, kind="ExternalInput")
    ndst = nc.dram_tensor("ndst", [128, NST], F32, kind="ExternalInput")
    w1_16 = nc.dram_tensor("w1_16", [F, H], F16, kind="ExternalInput")
    wmu_16 = nc.dram_tensor("wmu_16", [H, H], F16, kind="ExternalInput")
    wsig_16 = nc.dram_tensor("wsig_16", [H, H], F16, kind="ExternalInput")
    b1_rep = nc.dram_tensor("b1_rep", [128, H], F32, kind="ExternalInput")
    bmu_col = nc.dram_tensor("bmu_col", [H, 1], F32, kind="ExternalInput")
    bsig_col = nc.dram_tensor("bsig_col", [H, 1], F32, kind="ExternalInput")
    eye16_d = nc.dram_tensor("eye16", [128, 128], F16, kind="ExternalInput")
    ident_d = nc.dram_tensor("ident_t", [128, 128], TDT, kind="ExternalInput")
    iota_rep_d = nc.dram_tensor("iota_rep", [128, 128, NBMAX], F16,
                                kind="ExternalInput")
    eidx_d = nc.dram_tensor("eidx", [128, TOTCOLS], I16, kind="ExternalInput")
    dstloc_d = nc.dram_tensor("dstloc", [128, NSPB], F16, kind="ExternalInput")
    noise_t = nc.dram_tensor("noise_t", [H, SHARD], F16, kind="ExternalInput")
    z_out = nc.dram_tensor("z_out", [H, SHARD], F16, kind="ExternalOutput")
    dbg = bool(int(os.environ.get("KDBG", "0")))
    if dbg:
        x1_dbg = nc.dram_tensor("x1_dbg", [128, NST, TROW], TDT,
                                kind="ExternalOutput")
        h_dbg = nc.dram_tensor("h_dbg", [128, NST, TROW], TDT,
                               kind="ExternalOutput")

    x1_shard = nc.dram_tensor("x1_shard", [128, NST, TROW], TDT, kind="Internal")
    h_shard = nc.dram_tensor("h_shard", [128, NST, TROW], TDT, kind="Internal")
    x1_table = nc.dram_tensor("x1_table", [NPAD, TROW], TDT, kind="Internal",
                              addr_space="Shared")
    h_table = nc.dram_tensor("h_table", [NPAD, TROW], TDT, kind="Internal",
                             addr_space="Shared")
    cgroups = [list(range(NCORE))]

    spill_oh_dt = F16 if os.environ.get("KMIX", "1") == "1" else TDT

    with tile.TileContext(nc) as tc, ExitStack() as ctx:
        consts = ctx.enter_context(tc.tile_pool(name="consts", bufs=1))

        def cload(dram, shape, dtype, tag):
            t = consts.tile(shape, dtype, tag=tag)
            nc.sync.dma_start(t[:], dram[:])
            return t

        w1_sb = cload(w1_16, [F, H], F16, "w1")
        nsrc_sb = cload(nsrc, [128, NST], F32, "nsrc")

        # ---------------- phase 1: x1 = (feat*nsrc) @ W1 on own shard -------
        PG = 14  # physical supertiles per feat tile
        with tc.tile_pool(name="p1", bufs=3) as p1, \
             tc.tile_pool(name="p1ps", bufs=8, space="PSUM") as p1ps:
            for g0 in range(NST // PG):
                ftg = p1.tile([F, PG * 128], F16, tag="ftg")
                nc.sync.dma_start(ftg[:],
                                  feat_tt[:, g0 * PG * 128:(g0 + 1) * PG * 128])
                xg = p1.tile([128, PG, H], TDT, tag="xg")
                for si in range(PG):
                    s = g0 * PG + si
                    x1p = p1ps.tile([128, H], F32, tag="x1p")
                    nc.tensor.matmul(x1p[:], ftg[:, si * 128:(si + 1) * 128],
                                     w1_sb[:], start=True, stop=True)
                    nc.vector.tensor_scalar(xg[:, si, :], x1p[:],
                                            nsrc_sb[:, s:s + 1], None, ALU.mult)
                nc.sync.dma_start(
                    x1_shard[:, g0 * PG:(g0 + 1) * PG, 0:H], xg[:])
                if sim_mode:
                    # collective stand-in, pipelined per group
                    nc.sync.dma_start(
                        x1_table[0:SHARD, :]
                        .rearrange("(p s) e -> p s e", p=128)
                        [:, g0 * PG:(g0 + 1) * PG, :],
                        x1_shard[:, g0 * PG:(g0 + 1) * PG, :])

        # round-only consts: loaded after phase-1 emission
        wmu_sb = cload(wmu_16, [H, H], F16, "wmu")
        wsig_sb = cload(wsig_16, [H, H], F16, "wsig")
        ndst_sb = cload(ndst, [128, NST], F32, "ndst")
        nprod_sb = cload(nprod, [128, NST], F32, "nprod")
        b1_sb = cload(b1_rep, [128, H], F32, "b1")
        bmu_sb = cload(bmu_col, [H, 1], F32, "bmu")
        bsig_sb = cload(bsig_col, [H, 1], F32, "bsig")
        eye16 = cload(eye16_d, [128, 128], F16, "eye16")
        ident = cload(ident_d, [128, 128], TDT, "ident")
        iota_rep = cload(iota_rep_d, [128, 128, NBMAX], F16, "iota_rep")
        dstloc_sb = cload(dstloc_d, [128, NSPB], F16, "dstloc")
        eidx_sb = cload(eidx_d, [128, TOTCOLS], I16, "eidx")

        if dbg:
            nc.sync.dma_start(x1_dbg[:], x1_shard[:, :, :])
        if not sim_mode:
            nc.gpsimd.collective_compute("AllGather", ALU.bypass, cgroups,
                                         ins=[x1_shard[:]], outs=[x1_table[:]])

        HB = min(SB, 7)  # supertiles per PSUM bank

        # ---------------- message-passing round ----------------------------
        def mp_round(table, epilogue, pre_round, post_group, rtag):
            with tc.tile_pool(name=f"msgs{rtag}", bufs=3) as msgs, \
                 tc.tile_pool(name=f"ohp{rtag}", bufs=2) as ohp, \
                 tc.tile_pool(name=f"aggps{rtag}", bufs=2, space="PSUM") as aggps, \
                 tc.tile_pool(name=f"epi{rtag}", bufs=4) as epi, \
                 tc.tile_pool(name=f"episb{rtag}", bufs=4) as episb, \
                 tc.tile_pool(name=f"stg{rtag}", bufs=1) as stg, \
                 tc.tile_pool(name=f"epips{rtag}", bufs=2, space="PSUM") as epips, \
                 tc.tile_pool(name=f"epips2{rtag}", bufs=2, space="PSUM") as epips2:
                rctx = pre_round(stg)

                def run_epilogue(g, banks):
                    for j, s in enumerate(groups[g]):
                        agg = banks[j // HB][:, (j % HB) * H:(j % HB + 1) * H]
                        epilogue(g, j, s, agg, rctx, epi, episb, epips, epips2)
                    post_group(g, rctx)

                pending = None
                for g in range(NG):
                    sts = groups[g]
                    # emit window 0 last (its table rows overlap the own-shard
                    # copy; avoids Pool head-of-line blocking at the boundary)
                    mt = {}
                    for w in (1, 2, 3, 0):
                        r_gw = int(rows_gw[g, w])
                        if r_gw == 0:
                            continue
                        m = msgs.tile([128, ROWS_CAP, TB], TDT, tag=f"m{w}")
                        raw_gather(
                            nc.gpsimd, m[:, 0:r_gw, :],
                            table[w * WIN:(w + 1) * WIN, 0:TB],
                            eidx_sb[:, col_off[(g, w)]:
                                    col_off[(g, w)] + r_gw * (ST // 16)],
                            num_idxs=r_gw * ST, num_idxs_reg=r_gw * ST,
                            elem_size=TB, elem_step=TROW)
                        mt[w] = m
                    # previous group's epilogue issues while this group's
                    # gathers are in flight (PE queue is in-order: emitting it
                    # before the reduce matmuls keeps PE busy during the wait)
                    if pending is not None:
                        run_epilogue(*pending)
                        pending = None
                    # spill one-hots (DVE; independent of gathers)
                    ohs = {}
                    for w in range(NWIN):
                        nb = sum(n for (_, _, _, n, _) in spill_layout[(g, w)])
                        if nb == 0:
                            continue
                        b0 = spb_off[(g, w)]
                        oh = ohp.tile([128, 128, NBMAX], spill_oh_dt,
                                      tag=f"oh{w}")
                        nc.vector.tensor_tensor(
                            oh[:, :, 0:nb], iota_rep[:, :, 0:nb],
                            dstloc_sb[:, None, b0:b0 + nb]
                            .broadcast_to([128, 128, nb]),
                            ALU.is_equal)
                        ohs[w] = oh
                    # PSUM banks: HB supertiles each
                    nbank = -(-SB // HB)
                    banks = []
                    for hb in range(nbank):
                        nsi = min(HB, SB - hb * HB)
                        ab = aggps.tile([128, nsi * H], F32, tag=f"ab{hb}")
                        nc.vector.memset(ab[:], 0.0)
                        banks.append(ab)
                    # matmul op list, bank-major so bank 0's epilogues can
                    # start before bank 1's reduce finishes; last op per bank
                    # carries the stop flag
                    ops = []
                    for hb in range(nbank):
                        for w in range(NWIN):
                            for (k, nk, rs) in grid_layout[(g, w)]:
                                lo, hi = hb * HB, min((hb + 1) * HB, nk)
                                if lo < nk:
                                    ops.append(("g", w, hb, lo, hi, rs))
                            for (j, s, rs, nb, bloc) in spill_layout[(g, w)]:
                                if j // HB != hb:
                                    continue
                                for b in range(nb):
                                    ops.append(("s", w, hb, j, rs + b,
                                                bloc + b))
                    last_of = {}
                    for i, op in enumerate(ops):
                        last_of[op[2]] = i
                    for i, op in enumerate(ops):
                        stop = (last_of[op[2]] == i)
                        if op[0] == "g":
                            _, w, hb, lo, hi, rs = op
                            nc.tensor.matmul(
                                banks[hb][:, 0:(hi - lo) * H], ident[:],
                                mt[w][:, rs + lo:rs + hi, 0:H],
                                start=False, stop=stop)
                        else:
                            _, w, hb, j, row, bi = op
                            jj = j % HB
                            nc.tensor.matmul(
                                banks[hb][:, jj * H:(jj + 1) * H],
                                ohs[w][:, :, bi],
                                mt[w][:, row, 0:H],
                                start=False, stop=stop)
                    pending = (g, banks)
                if pending is not None:
                    run_epilogue(*pending)

        # ---------------- round 1: h ----------------------------------------
        def pre_r1(stg):
            hg = stg.tile([128, NST, H], TDT, tag="hg")
            return dict(hg=hg)

        def post_g1(g, rctx):
            nc.sync.dma_start(h_shard[:, g * SB:(g + 1) * SB, 0:H],
                              rctx["hg"][:, g * SB:(g + 1) * SB, :])
            if sim_mode:
                nc.sync.dma_start(
                    h_table[0:SHARD, :]
                    .rearrange("(p s) e -> p s e", p=128)
                    [:, g * SB:(g + 1) * SB, :],
                    h_shard[:, g * SB:(g + 1) * SB, :])

        def epi_round1(g, j, s, agg, rctx, epi, episb, epips, epips2):
            if cfg.get("B1Z", True):
                nc.scalar.activation(rctx["hg"][:, s, :], agg, ACTF.Relu,
                                     scale=nprod_sb[:, s:s + 1])
            else:
                hp = epi.tile([128, H], F32, tag="hp")
                nc.vector.scalar_tensor_tensor(hp[:], agg, ndst_sb[:, s:s + 1],
                                               b1_sb[:], ALU.mult, ALU.add)
                nc.scalar.activation(rctx["hg"][:, s, :], hp[:], ACTF.Relu,
                                     scale=nsrc_sb[:, s:s + 1])

        # ---------------- round 2: z -----------------------------------------
        def pre_r2(stg):
            zg = stg.tile([H, SHARD], F16, tag="zg")
            ng = stg.tile([H, SHARD], F16, tag="ng")
            nc.sync.dma_start(ng[:], noise_t[:])
            return dict(zg=zg, ng=ng)

        def post_g2(g, rctx):
            nc.sync.dma_start(z_out[:, g * SB * 128:(g + 1) * SB * 128],
                              rctx["zg"][:, g * SB * 128:(g + 1) * SB * 128])

        bz = cfg.get("BZ", True)  # b_mu == b_sig == 0 fast path

        def epi_round2(g, j, s, agg, rctx, epi, episb, epips, epips2):
            a2s = epi.tile([128, H], F16, tag="a2s")
            nc.scalar.activation(a2s[:], agg, ACTF.Identity,
                                 scale=ndst_sb[:, s:s + 1])
            a2tp = epips.tile([H, 128], F16, tag="a2tp")
            nc.tensor.matmul(a2tp[:], a2s[:], eye16[:], is_transpose=True)
            a2t = epi.tile([H, 128], F16, tag="a2t")
            nc.vector.tensor_scalar(a2t[:], a2tp[:], 1.0, None, ALU.mult)
            musg = epips2.tile([H, 2, 128], F32, tag="musg")
            nc.tensor.matmul(musg[:, 0, :], wmu_sb[:], a2t[:], start=True,
                             stop=True)
            nc.tensor.matmul(musg[:, 1, :], wsig_sb[:], a2t[:], start=True,
                             stop=True)
            es = episb.tile([H, 128], F32, tag="es")
            if bz:
                nc.scalar.activation(es[:], musg[:, 1, :], ACTF.Exp)
            else:
                nc.scalar.activation(es[:], musg[:, 1, :], ACTF.Exp,
                                     bias=bsig_sb[:])
            nz = episb.tile([H, 128], F32, tag="nz")
            nc.vector.scalar_tensor_tensor(
                nz[:], rctx["ng"][:, s * 128:(s + 1) * 128], 1.0, es[:],
                ALU.mult, ALU.mult)
            if bz:
                nc.vector.scalar_tensor_tensor(
                    rctx["zg"][:, s * 128:(s + 1) * 128], musg[:, 0, :], 0.0,
                    nz[:], ALU.add, ALU.add)
            else:
                mub = episb.tile([H, 128], F32, tag="mub")
                nc.scalar.activation(mub[:], musg[:, 0, :], ACTF.Identity,
                                     bias=bmu_sb[:])
                nc.vector.scalar_tensor_tensor(
                    rctx["zg"][:, s * 128:(s + 1) * 128], mub[:], 0.0, nz[:],
                    ALU.add, ALU.add)

        kphase = int(os.environ.get("KPHASE", "4"))
        if kphase >= 2:
            mp_round(x1_table, epi_round1, pre_r1, post_g1, "a")
        if kphase >= 3:
            if dbg:
                nc.sync.dma_start(h_dbg[:], h_shard[:, :, :])
            if not sim_mode:
                nc.gpsimd.collective_compute("AllGather", ALU.bypass, cgroups,
                                             ins=[h_shard[:]],
                                             outs=[h_table[:]])
        if kphase >= 4:
            mp_round(h_table, epi_round2, pre_r2, post_g2, "b")

    nc.finalize()
    return nc


def host_inputs(feat, src, dst, noise, W1, b1, W_mu, b_mu, W_sig, b_sig,
                cfg, plans, meta):
    N, NCORE, SHARD, NPAD = (cfg[k] for k in ("N", "NCORE", "SHARD", "NPAD"))
    NST, F, H, NWIN = cfg["NST"], cfg["F"], cfg["H"], cfg["NWIN"]
    NSPB = meta["NSPB"]
    pos_of = meta["pos_of"]
    feat = np.asarray(feat, dtype=np.float32)
    noise = np.asarray(noise, dtype=np.float32)
    src = np.asarray(src); dst = np.asarray(dst)

    deg_out = np.bincount(src, minlength=NPAD).astype(np.float32)
    deg_in = np.bincount(dst, minlength=NPAD).astype(np.float32)
    norm_src = np.maximum(deg_out, 1.0) ** -0.5
    norm_dst = np.maximum(deg_in, 1.0) ** -0.5

    inv = np.empty(NPAD, dtype=np.int64)
    inv[pos_of] = np.arange(NPAD)          # node at each position

    featp = np.zeros((NPAD, F), dtype=np.float32)
    featp[pos_of[:N]] = feat
    noisep = np.zeros((NPAD, H), dtype=np.float32)
    noisep[pos_of[:N]] = noise
    ns_p = norm_src[inv]
    nd_p = norm_dst[inv]

    TDT8 = os.environ.get("KDT", "f8") == "f8"
    eye16 = np.eye(128, dtype=np.float16)
    if TDT8:
        import ml_dtypes
        ident = np.eye(128).astype(ml_dtypes.float8_e4m3fn)
    else:
        ident = np.eye(128, dtype=np.float16)
    iota_rep = np.tile(np.arange(128, dtype=np.float16)[None, :, None],
                       (128, 1, meta["NBMAX"]))
    shared = dict(
        w1_16=np.asarray(W1, dtype=np.float16),
        wmu_16=np.asarray(W_mu, dtype=np.float16),
        wsig_16=np.asarray(W_sig, dtype=np.float16),
        b1_rep=np.tile(np.asarray(b1, dtype=np.float32)[None, :], (128, 1)),
        bmu_col=np.asarray(b_mu, dtype=np.float32).reshape(H, 1),
        bsig_col=np.asarray(b_sig, dtype=np.float32).reshape(H, 1),
        eye16=eye16, ident_t=ident, iota_rep=iota_rep,
    )
    in_maps = []
    for c in range(NCORE):
        lo, hi = c * SHARD, (c + 1) * SHARD
        m = dict(shared)
        m["feat_tt"] = featp[lo:hi].T.astype(np.float16).copy()
        m["nsrc"] = ns_p[lo:hi].reshape(NST, 128).T.copy()
        m["nprod"] = (ns_p * nd_p)[lo:hi].reshape(NST, 128).T.copy()
        m["ndst"] = nd_p[lo:hi].reshape(NST, 128).T.copy()
        m["noise_t"] = noisep[lo:hi].T.astype(np.float16).copy()
        m["eidx"] = plans[c]["eidx"]
        m["dstloc"] = plans[c]["dstloc"].astype(np.float16)
        in_maps.append(m)
    return in_maps


def run(feat, src, dst, noise, W1, b1, W_mu, b_mu, W_sig, b_sig,
        cfg=None, **spmd_kwargs):
    if cfg is None:
        cfg = default_cfg(feat.shape[0], src.shape[0], feat.shape[1],
                          W1.shape[1])
    cfg["B1Z"] = bool(np.all(np.asarray(b1) == 0.0))
    cfg["BZ"] = bool(np.all(np.asarray(b_mu) == 0.0)
                     and np.all(np.asarray(b_sig) == 0.0))
    plans, meta = build_plan(src, dst, cfg)
    nc = build_program(cfg, meta)
    in_maps = host_inputs(feat, src, dst, noise, W1, b1, W_mu, b_mu,
                          W_sig, b_sig, cfg, plans, meta)
    import time as _time
    last_exc = None
    for attempt in range(3):
        try:
            res = run_bass_kernel_spmd(nc, in_maps,
                                       core_ids=list(range(cfg["NCORE"])),
                                       **spmd_kwargs)
            break
        except Exception as e:
            last_exc = e
            _time.sleep(10.0)
    else:
        raise last_exc
    zs = [r["z_out"].T for r in res.results]          # [SHARD, H] each
    z_pos = np.concatenate(zs, axis=0)                # position-major
    z = z_pos[meta["pos_of"][:cfg["N"]]]
    return z.astype(np.float32), res


def kernel(feat, src, dst, noise, W1, b1, W_mu, b_mu, W_sig, b_sig):
    z, _ = run(feat, src, dst, noise, W1, b1, W_mu, b_mu, W_sig, b_sig)
    return z
